# revision 1
# baseline (speedup 1.0000x reference)
"""Biased MHSA Trainium2 kernel (8-core SPMD).

Sharding: core c -> (batch b = c//2, head-group g = c%2); each core computes
attention for 4 of the 8 heads of one batch and the partial output projection
for those heads. Host sums the two head-group partials per batch and adds bo.

Per-core device kernel (all in fp32, matmuls via float32r):
  - Q^T,K^T computed in [feature, token] layout; V in natural [token, feature]
    layout augmented with a ones column (gives softmax denominator for free).
  - S^T[k,q] = K_h^T.T @ (Q_h^T/8) per 128-key tile; DVE adds bias^T tile;
    ACT exp in place; U_aug^T[d|1, q] += V_aug.T @ P^T accumulated over keys.
  - Deferred softmax normalization: A^T = U^T * (1/r) with r broadcast across
    partitions via a 0-stride DMA; + bv.
  - O[tok, 512] = A^T.T @ wo accumulated over the 4 heads.
"""

import sys

if "/opt/trn_rl_repo" not in sys.path:
    sys.path.insert(0, "/opt/trn_rl_repo")

from contextlib import ExitStack

import numpy as np

import concourse.bass as bass
from concourse import bacc
import concourse.tile as tile
from concourse import mybir
from concourse.bass_utils import run_bass_kernel_spmd
from concourse.masks import make_identity

B, N, D = 4, 2048, 512
H, DH = 8, 64
HG = 4  # heads per core
GD = HG * DH  # 256 features per core
P = 128
QQ = 512  # q processed in chunks of 512
NKT = N // P  # 16 key tiles
NQQ = N // QQ  # 4 q chunks
NTOK = N // P  # 16 token tiles
KC = D // P  # 4 contraction chunks for projections
F32 = mybir.dt.float32
F32R = mybir.dt.float32r


def build_program():
    nc = bacc.Bacc("TRN2", target_bir_lowering=False)
    xT = nc.dram_tensor("xT", [D, N], F32R, kind="ExternalInput")
    biasT = nc.dram_tensor("biasT", [N, N], F32R, kind="ExternalInput")
    wq = nc.dram_tensor("wq", [D, GD], F32R, kind="ExternalInput")
    wk = nc.dram_tensor("wk", [D, GD], F32R, kind="ExternalInput")
    wv = nc.dram_tensor("wv", [D, GD], F32R, kind="ExternalInput")
    wo = nc.dram_tensor("wo", [GD, D], F32R, kind="ExternalInput")
    bq = nc.dram_tensor("bq", [GD], F32, kind="ExternalInput")
    bk = nc.dram_tensor("bk", [GD], F32, kind="ExternalInput")
    bv = nc.dram_tensor("bv", [GD], F32, kind="ExternalInput")
    out = nc.dram_tensor("out", [N, D], F32, kind="ExternalOutput")

    with tile.TileContext(nc) as tc, ExitStack() as ctx:
        const = ctx.enter_context(tc.tile_pool(name="const", bufs=1))
        big = ctx.enter_context(tc.tile_pool(name="big", bufs=1))
        bias_pool = ctx.enter_context(tc.tile_pool(name="biasp", bufs=5))
        p_pool = ctx.enter_context(tc.tile_pool(name="probs", bufs=4))
        small = ctx.enter_context(tc.tile_pool(name="small", bufs=2))
        o_pool = ctx.enter_context(tc.tile_pool(name="outp", bufs=2))
        psum_mm = ctx.enter_context(tc.tile_pool(name="psum_mm", bufs=2, space="PSUM"))
        psum_u = ctx.enter_context(tc.tile_pool(name="psum_u", bufs=1, space="PSUM"))
        dram_p = ctx.enter_context(tc.tile_pool(name="dram_p", bufs=2, space="DRAM"))

        # ---- load inputs ----
        xT_s = big.tile([P, KC, N], F32R)  # x^T as [128, kc, tok]
        nc.sync.dma_start(out=xT_s, in_=xT.rearrange("(kc p) n -> p kc n", p=P))
        wq_s = const.tile([P, KC, GD], F32R)
        nc.sync.dma_start(out=wq_s, in_=wq.rearrange("(kc p) f -> p kc f", p=P))
        wk_s = const.tile([P, KC, GD], F32R)
        nc.sync.dma_start(out=wk_s, in_=wk.rearrange("(kc p) f -> p kc f", p=P))
        wv_s = const.tile([P, KC, GD], F32R)
        nc.sync.dma_start(out=wv_s, in_=wv.rearrange("(kc p) f -> p kc f", p=P))
        wo_s = const.tile([DH, HG, D], F32R)  # per-head wo rows: [64, h, 512]
        nc.sync.dma_start(out=wo_s, in_=wo.rearrange("(h p) d -> p h d", p=DH))
        bq_s = const.tile([P, 2], F32)
        nc.sync.dma_start(out=bq_s, in_=bq.rearrange("(fc p) -> p fc", p=P))
        bk_s = const.tile([P, 2], F32)
        nc.sync.dma_start(out=bk_s, in_=bk.rearrange("(fc p) -> p fc", p=P))
        bqs = const.tile([P, 2], F32)  # bq * 0.125 (scale folded into Q)
        nc.vector.tensor_scalar_mul(bqs, bq_s, 0.125)
        ident_f = const.tile([P, P], F32)
        make_identity(nc, ident_f)
        ident = const.tile([P, P], F32R)
        nc.vector.tensor_copy(ident, ident_f)

        # ---- projections ----
        # Q^T, K^T: [128, fc, tok] (feature on partitions; head h lives at
        # partitions (h%2)*64..+64 of chunk fc=h//2)
        qT = big.tile([P, 2, N], F32R)
        kT = big.tile([P, 2, N], F32R)
        for name, w_s, dst, b_ap, scale in (
            ("q", wq_s, qT, bqs, 0.125),
            ("k", wk_s, kT, bk_s, 1.0),
        ):
            for fc in range(2):
                for nn in range(NQQ):
                    ps = psum_mm.tile([P, QQ], F32, tag="mm")
                    for kc in range(KC):
                        nc.tensor.matmul(
                            ps,
                            (w_s[:, kc, fc * P : (fc + 1) * P]),
                            (xT_s[:, kc, nn * QQ : (nn + 1) * QQ]),
                            start=(kc == 0),
                            stop=(kc == KC - 1),
                        )
                    # (x@w + b) * scale  ==  psum*scale + b*scale
                    nc.vector.tensor_scalar(
                        dst[:, fc, nn * QQ : (nn + 1) * QQ],
                        ps,
                        scale,
                        b_ap[:, fc : fc + 1],
                        op0=mybir.AluOpType.mult,
                        op1=mybir.AluOpType.add,
                    )

        # V natural layout, augmented ones column: vaug[128tok, h, kt, 65]
        vaug = big.tile([P, HG, NKT, DH + 1], F32R)
        nc.vector.memset(vaug[:, :, :, DH : DH + 1].bitcast(F32), 1.0)
        for kt in range(NKT):
            ps = psum_mm.tile([P, GD], F32, tag="mm")
            for kc in range(KC):
                nc.tensor.matmul(
                    ps,
                    (xT_s[:, kc, kt * P : (kt + 1) * P]),
                    (wv_s[:, kc, :]),
                    start=(kc == 0),
                    stop=(kc == KC - 1),
                )
            # scatter heads into vaug[:, h, kt, 0:64] (bv folded in later)
            nc.vector.tensor_copy(
                vaug[:, :, kt, 0:DH],
                ps.rearrange("p (h d) -> p h d", h=HG),
            )

        # ---- attention ----
        # head-pairs in flight (PSUM: 2x ps[128,QW] + 2x pu[65,QW] = 8 banks),
        # q in QW=1024 chunks so each LDWEIGHTS serves 2 matmuls. The bias add
        # runs on the PE (identity-matmul accumulated into the scores PSUM
        # group) for PE_BIAS_EVERY-th k-tile to keep the PE dense (HAM), on
        # DVE (in-place on PSUM) otherwise.
        QW = 1024
        NQW = N // QW
        NSL = QW // QQ  # 512-wide matmul slices per chunk
        PE_BIAS_EVERY = 8
        aT = big.tile([DH, HG, N], F32R)  # A^T = normalized attn out, [64, h, tok]

        def normalize_head(h, pu_h, qsl):
            # Evict U_aug^T fast (releases the PSUM bank for the next pass);
            # normalization runs off the critical path from SBUF. bv is folded
            # into the host epilogue (out += bv @ wo).
            uT = small.tile([DH + 1, QW], F32, tag="uT", name="uT")
            nc.vector.tensor_copy(uT, pu_h)
            r_d = dram_p.tile([QW], F32, tag="rd", name="r_d")
            nc.gpsimd.dma_start(out=r_d[:], in_=uT[DH : DH + 1, :])
            r128 = small.tile([P, QW // P], F32, tag="r128", name="r128")
            nc.gpsimd.dma_start(out=r128, in_=r_d[:].rearrange("(f p) -> p f", p=P))
            nc.vector.reciprocal(r128, r128)
            rd2 = dram_p.tile([QW], F32, tag="rd2", name="rd2")
            nc.gpsimd.dma_start(out=rd2[:].rearrange("(f p) -> p f", p=P), in_=r128)
            bc = small.tile([DH, QW], F32, tag="bc", name="bc")
            rap = rd2[:]
            nc.gpsimd.dma_start(
                out=bc,
                in_=bass.AP(
                    tensor=rap.tensor, offset=rap.offset,
                    ap=[[0, DH]] + list(rap.ap),
                ),
            )
            nc.vector.tensor_tensor(
                aT[:, h, qsl], uT[0:DH, :], bc, op=mybir.AluOpType.mult
            )

        def oproj_tile(t):
            # O[tok, 512] = sum_h A_h @ wo_h for one 128-token tile
            ps = psum_mm.tile([P, D], F32, tag="mm", name="ps_o")
            for h in range(HG):
                nc.tensor.matmul(
                    ps,
                    aT[:, h, t * P : (t + 1) * P],
                    wo_s[:, h, :],
                    start=(h == 0),
                    stop=(h == HG - 1),
                )
            ob = o_pool.tile([P, D], F32, name="ob")
            nc.vector.tensor_copy(ob, ps)
            nc.sync.dma_start(out=out[t * P : (t + 1) * P, :], in_=ob)

        for q2 in range(NQW):  # q2 outer so O-proj of q2 overlaps q2+1
            for hp in range(2):  # head pair: heads (2hp, 2hp+1), both in fc=hp
                qsl = slice(q2 * QW, (q2 + 1) * QW)
                pu = [
                    psum_u.tile([DH + 1, QW], F32, tag=f"u{i}", name=f"pu{i}")
                    for i in range(2)
                ]
                for kt in range(NKT):
                    bt = bias_pool.tile([P, QW], F32R, name="bt")
                    for j in range(4):
                        nc.sync.dma_start(
                            out=bt[:, j * (QW // 4) : (j + 1) * (QW // 4)],
                            in_=biasT[
                                kt * P : (kt + 1) * P,
                                q2 * QW + j * (QW // 4) :
                                q2 * QW + (j + 1) * (QW // 4),
                            ],
                        )
                    pe_bias = (kt % PE_BIAS_EVERY) == PE_BIAS_EVERY - 1
                    for i in range(2):
                        h = 2 * hp + i
                        ho = i * DH
                        ps = psum_mm.tile([P, QW], F32, tag="mm", name="ps")
                        for sl in range(NSL):
                            ssl = slice(sl * QQ, (sl + 1) * QQ)
                            if pe_bias:
                                nc.tensor.matmul(
                                    ps[:, ssl], ident, bt[:, ssl],
                                    start=True, stop=False,
                                )
                            nc.tensor.matmul(
                                ps[:, ssl],
                                kT[ho : ho + DH, hp, kt * P : (kt + 1) * P],
                                qT[ho : ho + DH, hp, q2 * QW + sl * QQ :
                                   q2 * QW + (sl + 1) * QQ],
                                start=not pe_bias,
                                stop=True,
                            )
                        sp = p_pool.tile([P, QW], F32R, tag="sp", name="sp")
                        if not pe_bias:
                            sc = p_pool.tile([P, QW], F32, tag="sc", name="sc")
                            for sl in range(NSL):
                                ssl = slice(sl * QQ, (sl + 1) * QQ)
                                nc.vector.tensor_tensor(
                                    sc[:, ssl], ps[:, ssl],
                                    bt.bitcast(F32)[:, ssl],
                                    op=mybir.AluOpType.add,
                                )
                            nc.scalar.activation(
                                sp, sc, mybir.ActivationFunctionType.Exp
                            )
                        else:
                            nc.scalar.activation(
                                sp, ps, mybir.ActivationFunctionType.Exp
                            )
                        for sl in range(NSL):
                            ssl = slice(sl * QQ, (sl + 1) * QQ)
                            nc.tensor.matmul(
                                pu[i][:, ssl],
                                vaug[:, h, kt, :],
                                sp[:, ssl],
                                start=(kt == 0),
                                stop=(kt == NKT - 1),
                            )
                    if kt == NKT - 1:
                        for i in range(2):
                            normalize_head(2 * hp + i, pu[i], qsl)
            # output projection for the PREVIOUS q chunk: its aT deps are long
            # resolved, so the in-order PE queue never stalls on the
            # normalize-chain latency.
            if q2 > 0:
                for t in range((q2 - 1) * QW // P, q2 * QW // P):
                    oproj_tile(t)
        for t in range((NQW - 1) * QW // P, NQW * QW // P):
            oproj_tile(t)

    nc.compile()
    return nc


_NC = None


def _get_nc():
    global _NC
    if _NC is None:
        _NC = build_program()
    return _NC


def make_in_maps(x, attn_bias, wq, bq, wk, bk, wv, bv, wo, bo):
    x = np.asarray(x, np.float32)
    attn_bias = np.asarray(attn_bias, np.float32)
    in_maps = []
    for c in range(8):
        b, g = c // 2, c % 2
        sl = slice(g * GD, (g + 1) * GD)
        in_maps.append(
            {
                "xT": np.ascontiguousarray(x[b].T),
                "biasT": np.ascontiguousarray(attn_bias[b, 0].T),
                "wq": np.ascontiguousarray(np.asarray(wq, np.float32)[:, sl]),
                "wk": np.ascontiguousarray(np.asarray(wk, np.float32)[:, sl]),
                "wv": np.ascontiguousarray(np.asarray(wv, np.float32)[:, sl]),
                "wo": np.ascontiguousarray(np.asarray(wo, np.float32)[sl, :]),
                "bq": np.ascontiguousarray(np.asarray(bq, np.float32)[sl]),
                "bk": np.ascontiguousarray(np.asarray(bk, np.float32)[sl]),
                "bv": np.ascontiguousarray(np.asarray(bv, np.float32)[sl]),
            }
        )
    return in_maps


def gather_output(results, bo, bv, wo):
    bo = np.asarray(bo, np.float32)
    row = bo + np.asarray(bv, np.float32) @ np.asarray(wo, np.float32)
    out = np.empty((B, N, D), np.float32)
    for b in range(B):
        out[b] = results[2 * b]["out"] + results[2 * b + 1]["out"] + row[None, :]
    return out


def kernel(x, attn_bias, wq, bq, wk, bk, wv, bv, wo, bo, _trace=False):
    nc = _get_nc()
    in_maps = make_in_maps(x, attn_bias, wq, bq, wk, bk, wv, bv, wo, bo)
    res = run_bass_kernel_spmd(nc, in_maps, core_ids=list(range(8)), trace=_trace)
    out = gather_output(res.results, bo, bv, wo)
    if _trace:
        kernel.last_results = res
    return out



# revision 2
# speedup vs baseline: 1.4094x; 1.4094x over previous
"""Biased MHSA Trainium2 kernel (8-core SPMD), v2.

Sharding: core c -> (batch b = c//2, head-group g = c%2); each core computes
attention for 4 of the 8 heads of one batch and the partial output projection
for those heads. Host sums the two head-group partials per batch and adds
bo + bv @ wo (bv folded via softmax row-sum = 1; bk dropped entirely since a
per-query constant shift cancels in softmax).

v2 design (vs v1):
  - exp(S + bias) = exp(S) * exp(bias): exp(bias) is precomputed on HOST and
    shipped as bf16 (halves bias HBM traffic to 8.4 MB/core and turns the
    fp32-PSUM DVE bias-add into a bf16x bf16 multiply at 2x DVE rate).
  - ACT reads scores PSUM directly: one exp per head-PAIR on a [128,1024]
    PSUM tile (amortizes the 172-cycle PSUM access), double-buffered.
  - PV accumulates per head into 4 single-bank [65,512] PSUM tiles (bf16 V
    with a ones column giving the softmax denominator for free).
  - O-proj contracts 128 features/pass via 2-head-packed A^T (halves O rows).
  - PE emission: PV for key-tile kt-1 is interleaved after the score MMs of
    kt, so the PE never waits on the exp->mult chain.
"""

import sys

if "/opt/trn_rl_repo" not in sys.path:
    sys.path.insert(0, "/opt/trn_rl_repo")

from contextlib import ExitStack

import numpy as np
import ml_dtypes

import concourse.bass as bass
from concourse import bacc
import concourse.tile as tile
from concourse import mybir
from concourse.bass_utils import run_bass_kernel_spmd

B, N, D = 4, 2048, 512
H, DH = 8, 64
HG = 4  # heads per core
GD = HG * DH  # 256 features per core
P = 128
QC = 512  # q processed in chunks of 512
NQC = N // QC  # 4 q chunks
NKT = N // P  # 16 key tiles
KC = D // P  # 4 contraction chunks for projections
F32 = mybir.dt.float32
F32R = mybir.dt.float32r
BF16 = mybir.dt.bfloat16


def build_program():
    nc = bacc.Bacc("TRN2", target_bir_lowering=False)
    xT = nc.dram_tensor("xT", [D, N], F32R, kind="ExternalInput")
    ebT = nc.dram_tensor("ebT", [N, N], BF16, kind="ExternalInput")  # exp(bias)^T
    wq = nc.dram_tensor("wq", [D, GD], F32R, kind="ExternalInput")
    wk = nc.dram_tensor("wk", [D, GD], F32R, kind="ExternalInput")
    wv = nc.dram_tensor("wv", [D, GD], F32R, kind="ExternalInput")
    wo = nc.dram_tensor("wo", [GD, D], F32R, kind="ExternalInput")
    bq = nc.dram_tensor("bq", [GD], F32, kind="ExternalInput")
    out = nc.dram_tensor("out", [N, D], F32, kind="ExternalOutput")

    with tile.TileContext(nc) as tc, ExitStack() as ctx:
        const = ctx.enter_context(tc.tile_pool(name="const", bufs=1))
        big = ctx.enter_context(tc.tile_pool(name="big", bufs=1))
        et_pool = ctx.enter_context(tc.tile_pool(name="etp", bufs=4))
        sp_pool = ctx.enter_context(tc.tile_pool(name="spp", bufs=3))
        spb_pool = ctx.enter_context(tc.tile_pool(name="spbp", bufs=3))
        u_pool = ctx.enter_context(tc.tile_pool(name="up", bufs=2))
        r_pool = ctx.enter_context(tc.tile_pool(name="rp", bufs=2))
        o_pool = ctx.enter_context(tc.tile_pool(name="op", bufs=2))
        psum_s = ctx.enter_context(tc.tile_pool(name="psum_s", bufs=2, space="PSUM"))
        psum_u = ctx.enter_context(tc.tile_pool(name="psum_u", bufs=1, space="PSUM"))
        dram_p = ctx.enter_context(tc.tile_pool(name="dram_p", bufs=2, space="DRAM"))

        # ---- load inputs (weights first; x in token chunks so proj starts early)
        wq_s = const.tile([P, KC, GD], F32R)
        nc.sync.dma_start(out=wq_s, in_=wq.rearrange("(kc p) f -> p kc f", p=P))
        wk_s = const.tile([P, KC, GD], F32R)
        nc.sync.dma_start(out=wk_s, in_=wk.rearrange("(kc p) f -> p kc f", p=P))
        wv_s = const.tile([P, KC, GD], F32R)
        nc.sync.dma_start(out=wv_s, in_=wv.rearrange("(kc p) f -> p kc f", p=P))
        # wo rows packed 2 heads per 128: wo2[p, j, :] = wo[j*128 + p, :]
        wo2_s = const.tile([P, 2, D], F32R)
        nc.sync.dma_start(out=wo2_s, in_=wo.rearrange("(j p) d -> p j d", p=P))
        bq_s = const.tile([P, 2], F32)
        nc.sync.dma_start(out=bq_s, in_=bq.rearrange("(fc p) -> p fc", p=P))
        bqs = const.tile([P, 2], F32)  # bq * 0.125 (scale folded into Q)
        nc.vector.tensor_scalar_mul(bqs, bq_s, 0.125)

        xT_s = big.tile([P, KC, N], F32R)  # x^T as [128, kc, tok]
        xT_r = xT.rearrange("(kc p) n -> p kc n", p=P)
        for c in range(NQC):
            csl = slice(c * QC, (c + 1) * QC)
            nc.sync.dma_start(out=xT_s[:, :, csl], in_=xT_r[:, :, csl])

        # ---- projections ----
        # Q^T, K^T: [128, fc, tok]; head h lives at partitions (h%2)*64 of
        # chunk fc=h//2 (so head pair j=(2j,2j+1) occupies all of fc=j).
        qT = big.tile([P, 2, N], F32R)
        kT = big.tile([P, 2, N], F32R)
        # V natural layout, bf16, augmented ones column: vaug[128tok, h, kt, 65]
        vaug = big.tile([P, HG, NKT, DH + 1], BF16)
        nc.vector.memset(vaug[:, :, :, DH : DH + 1], 1.0)

        for c in range(NQC):
            csl = slice(c * QC, (c + 1) * QC)
            for fc in range(2):
                ps = psum_s.tile([P, QC], F32, tag="s")
                for kc in range(KC):
                    nc.tensor.matmul(
                        ps,
                        wk_s[:, kc, fc * P : (fc + 1) * P],
                        xT_s[:, kc, csl],
                        start=(kc == 0),
                        stop=(kc == KC - 1),
                    )
                nc.vector.tensor_copy(kT[:, fc, csl], ps)
            for kt in range(4 * c, 4 * c + 4):
                ps = psum_s.tile([P, GD], F32, tag="s")
                for kc in range(KC):
                    nc.tensor.matmul(
                        ps,
                        xT_s[:, kc, kt * P : (kt + 1) * P],
                        wv_s[:, kc, :],
                        start=(kc == 0),
                        stop=(kc == KC - 1),
                    )
                nc.vector.tensor_copy(
                    vaug[:, :, kt, 0:DH],
                    ps.rearrange("p (h d) -> p h d", h=HG),
                )
            for fc in range(2):
                ps = psum_s.tile([P, QC], F32, tag="s")
                for kc in range(KC):
                    nc.tensor.matmul(
                        ps,
                        wq_s[:, kc, fc * P : (fc + 1) * P],
                        xT_s[:, kc, csl],
                        start=(kc == 0),
                        stop=(kc == KC - 1),
                    )
                # (x@wq + bq) * 0.125 == psum*0.125 + bq*0.125
                nc.vector.tensor_scalar(
                    qT[:, fc, csl],
                    ps,
                    0.125,
                    bqs[:, fc : fc + 1],
                    op0=mybir.AluOpType.mult,
                    op1=mybir.AluOpType.add,
                )

        # ---- attention ----
        # A^T 2-head-packed: aT2[p, j, q]; partitions 0:64 = head 2j,
        # 64:128 = head 2j+1 (matches wo2_s packing).
        aT2 = big.tile([P, 2, N], F32R)

        def et_dma(qc, kt):
            # exp(bias)^T tile [128 keys, 512 q], split in 2 for queue overlap
            bt = et_pool.tile([P, QC], BF16, tag="et", name="et")
            for j in range(2):
                hw = QC // 2
                nc.sync.dma_start(
                    out=bt[:, j * hw : (j + 1) * hw],
                    in_=ebT[
                        kt * P : (kt + 1) * P,
                        qc * QC + j * hw : qc * QC + (j + 1) * hw,
                    ],
                )
            return bt

        def normalize(qc, pu):
            # evict U (frees PSUM), extract r = row 64, 1/r, broadcast via
            # DRAM roundtrip, write normalized A^T (2-head packed).
            qsl = slice(qc * QC, (qc + 1) * QC)
            uts = []
            rd = dram_p.tile([HG, QC], F32, tag="rd", name="rd")
            for h in range(HG):
                ut = u_pool.tile([DH + 1, QC], F32, tag=f"ut{h}", name=f"ut{h}")
                nc.vector.tensor_copy(ut, pu[h])
                nc.gpsimd.dma_start(out=rd[h : h + 1, :], in_=ut[DH : DH + 1, :])
                uts.append(ut)
            rr = r_pool.tile([HG, QC], F32, tag="rr", name="rr")
            nc.gpsimd.dma_start(out=rr, in_=rd[:, :])
            rc = r_pool.tile([HG, QC], F32, tag="rc", name="rc")
            nc.vector.reciprocal(rc, rr)
            rd2 = dram_p.tile([HG, QC], F32, tag="rd2", name="rd2")
            nc.gpsimd.dma_start(out=rd2[:, :], in_=rc)
            rb = r_pool.tile([DH, HG, QC], F32, tag="rb", name="rb")
            rap = rd2[:, :]
            nc.gpsimd.dma_start(
                out=rb,
                in_=bass.AP(
                    tensor=rap.tensor, offset=rap.offset,
                    ap=[[0, DH]] + list(rap.ap),
                ),
            )
            for h in range(HG):
                po = (h % 2) * DH
                nc.vector.tensor_tensor(
                    aT2[po : po + DH, h // 2, qsl],
                    uts[h][0:DH, :],
                    rb[:, h, :],
                    op=mybir.AluOpType.mult,
                )

        def oproj_tile(t):
            # O[tok, 512] = sum_j aT2[:, j, tsl].T @ wo2[:, j, :]
            tsl = slice(t * P, (t + 1) * P)
            po = psum_s.tile([P, D], F32, tag="s", name="po")
            for j in range(2):
                nc.tensor.matmul(
                    po, aT2[:, j, tsl], wo2_s[:, j, :],
                    start=(j == 0), stop=(j == 1),
                )
            ob = o_pool.tile([P, D], F32, name="ob")
            nc.vector.tensor_copy(ob, po)
            nc.sync.dma_start(out=out[tsl, :], in_=ob)

        for qc in range(NQC):
            qsl = slice(qc * QC, (qc + 1) * QC)
            pu = [
                psum_u.tile([DH + 1, QC], F32, tag=f"u{h}", name=f"pu{h}")
                for h in range(HG)
            ]
            bts = {}
            for kt in range(2):
                bts[kt] = et_dma(qc, kt)
            spb_hold = [None, None]
            for kt in range(NKT):
                if kt + 2 < NKT:
                    bts[kt + 2] = et_dma(qc, kt + 2)
                bt = bts.pop(kt)
                for j in range(2):  # head pair j = heads (2j, 2j+1)
                    ps2 = psum_s.tile([P, 2 * QC], F32, tag="s", name="ps2")
                    for i in range(2):
                        ho = i * DH
                        nc.tensor.matmul(
                            ps2[:, i * QC : (i + 1) * QC],
                            kT[ho : ho + DH, j, kt * P : (kt + 1) * P],
                            qT[ho : ho + DH, j, qsl],
                            start=True,
                            stop=True,
                        )
                    # PE: PV for the PREVIOUS kt right after this pair's
                    # score MMs — exp/mult of kt-1 are long done by now.
                    if kt > 0:
                        sprev = spb_hold[j]
                        for i in range(2):
                            nc.tensor.matmul(
                                pu[2 * j + i],
                                vaug[:, 2 * j + i, kt - 1, :],
                                sprev[:, i * QC : (i + 1) * QC],
                                start=(kt - 1 == 0),
                                stop=False,
                            )
                    sp = sp_pool.tile([P, 2 * QC], BF16, tag="sp", name="sp")
                    nc.scalar.activation(sp, ps2, mybir.ActivationFunctionType.Exp)
                    spb = spb_pool.tile([P, 2 * QC], BF16, tag="spb", name="spb")
                    for i in range(2):
                        isl = slice(i * QC, (i + 1) * QC)
                        nc.vector.tensor_tensor(
                            spb[:, isl], sp[:, isl], bt, op=mybir.AluOpType.mult
                        )
                    spb_hold[j] = spb
            for j in range(2):  # PV for kt = NKT-1
                sprev = spb_hold[j]
                for i in range(2):
                    nc.tensor.matmul(
                        pu[2 * j + i],
                        vaug[:, 2 * j + i, NKT - 1, :],
                        sprev[:, i * QC : (i + 1) * QC],
                        start=False,
                        stop=True,
                    )
            normalize(qc, pu)

        # output projection at the end (aT2 fully resolved; PSUM free)
        for t in range(N // P):
            oproj_tile(t)

    nc.compile()
    return nc


_NC = None


def _get_nc():
    global _NC
    if _NC is None:
        _NC = build_program()
    return _NC


def make_in_maps(x, attn_bias, wq, bq, wk, bk, wv, bv, wo, bo):
    x = np.asarray(x, np.float32)
    attn_bias = np.asarray(attn_bias, np.float32)
    # exp(bias)^T per batch, bf16 (shared by the 2 cores of each batch)
    ebTs = [
        np.exp(attn_bias[b, 0].T).astype(ml_dtypes.bfloat16) for b in range(B)
    ]
    xTs = [np.ascontiguousarray(x[b].T) for b in range(B)]
    in_maps = []
    for c in range(8):
        b, g = c // 2, c % 2
        sl = slice(g * GD, (g + 1) * GD)
        in_maps.append(
            {
                "xT": xTs[b],
                "ebT": ebTs[b],
                "wq": np.ascontiguousarray(np.asarray(wq, np.float32)[:, sl]),
                "wk": np.ascontiguousarray(np.asarray(wk, np.float32)[:, sl]),
                "wv": np.ascontiguousarray(np.asarray(wv, np.float32)[:, sl]),
                "wo": np.ascontiguousarray(np.asarray(wo, np.float32)[sl, :]),
                "bq": np.ascontiguousarray(np.asarray(bq, np.float32)[sl]),
            }
        )
    return in_maps


def gather_output(results, bo, bv, wo):
    bo = np.asarray(bo, np.float32)
    row = bo + np.asarray(bv, np.float32) @ np.asarray(wo, np.float32)
    out = np.empty((B, N, D), np.float32)
    for b in range(B):
        out[b] = results[2 * b]["out"] + results[2 * b + 1]["out"] + row[None, :]
    return out


def kernel(x, attn_bias, wq, bq, wk, bk, wv, bv, wo, bo, _trace=False):
    nc = _get_nc()
    in_maps = make_in_maps(x, attn_bias, wq, bq, wk, bk, wv, bv, wo, bo)
    res = run_bass_kernel_spmd(nc, in_maps, core_ids=list(range(8)), trace=_trace)
    out = gather_output(res.results, bo, bv, wo)
    if _trace:
        kernel.last_results = res
    return out


# revision 3
# speedup vs baseline: 1.6292x; 1.1559x over previous
"""Biased MHSA Trainium2 kernel (8-core SPMD), v3.

Sharding: core c -> (batch b = c//2, head-group g = c%2); each core computes
attention for 4 of the 8 heads of one batch and the partial output projection
for those heads. Host sums the two head-group partials per batch and adds
bo + bv @ wo (bv folded via softmax row-sum = 1; bk dropped entirely since a
per-query constant shift cancels in softmax).

Key structure:
  - exp(S + bias) = exp(S) * exp(bias): exp(bias) precomputed on HOST, bf16.
  - One ACT exp per head-pair on a [128,1024] scores PSUM tile (double
    buffered); DVE multiplies by exp(bias) in bf16 2x-packed mode.
  - PV accumulates into 4 single-bank [65,512] PSUM tiles (bf16 V + ones
    column = softmax denominator). PE emission interleaves PV of kt-1 after
    the score MMs of kt so the PE never waits on the exp->mult chain.
  - Projections for token chunks 1-3 are interleaved into attention chunk
    0's key loop (PE would otherwise idle cold at half HAM clock).
  - Softmax normalization is split: at the chunk boundary only the PSUM
    eviction + r-row DMAs are emitted (frees the accumulators); the
    reciprocal / broadcast / A^T writes are deferred into the next chunk's
    key loop so the in-order DVE queue never stalls the pipeline.
  - O-proj contracts 128 features/pass via 2-head-packed A^T, all at the
    end (tiles 0-11 emitted before the last normalize chain to keep PE hot).
"""

import sys

if "/opt/trn_rl_repo" not in sys.path:
    sys.path.insert(0, "/opt/trn_rl_repo")

from contextlib import ExitStack

import numpy as np
import ml_dtypes

import concourse.bass as bass
from concourse import bacc
import concourse.tile as tile
from concourse import mybir
from concourse.bass_utils import run_bass_kernel_spmd

B, N, D = 4, 2048, 512
H, DH = 8, 64
HG = 4  # heads per core
GD = HG * DH  # 256 features per core
P = 128
QC = 512  # q processed in chunks of 512
NQC = N // QC  # 4 q chunks
NKT = N // P  # 16 key tiles
KC = D // P  # 4 contraction chunks for projections
F32 = mybir.dt.float32
F32R = mybir.dt.float32r
BF16 = mybir.dt.bfloat16


def build_program():
    nc = bacc.Bacc("TRN2", target_bir_lowering=False)
    xT = nc.dram_tensor("xT", [D, N], F32R, kind="ExternalInput")
    ebT = nc.dram_tensor("ebT", [N, N], BF16, kind="ExternalInput")  # exp(bias)^T
    wq = nc.dram_tensor("wq", [D, GD], F32R, kind="ExternalInput")
    wk = nc.dram_tensor("wk", [D, GD], F32R, kind="ExternalInput")
    wv = nc.dram_tensor("wv", [D, GD], F32R, kind="ExternalInput")
    wo = nc.dram_tensor("wo", [GD, D], F32R, kind="ExternalInput")
    bq = nc.dram_tensor("bq", [GD], F32, kind="ExternalInput")
    out = nc.dram_tensor("out", [N, D], F32, kind="ExternalOutput")

    with tile.TileContext(nc) as tc, ExitStack() as ctx:
        const = ctx.enter_context(tc.tile_pool(name="const", bufs=1))
        big = ctx.enter_context(tc.tile_pool(name="big", bufs=1))
        et_pool = ctx.enter_context(tc.tile_pool(name="etp", bufs=4))
        sp_pool = ctx.enter_context(tc.tile_pool(name="spp", bufs=3))
        spb_pool = ctx.enter_context(tc.tile_pool(name="spbp", bufs=3))
        u_pool = ctx.enter_context(tc.tile_pool(name="up", bufs=2))
        r_pool = ctx.enter_context(tc.tile_pool(name="rp", bufs=2))
        o_pool = ctx.enter_context(tc.tile_pool(name="op", bufs=3))
        psum_s = ctx.enter_context(tc.tile_pool(name="psum_s", bufs=2, space="PSUM"))
        psum_u = ctx.enter_context(tc.tile_pool(name="psum_u", bufs=1, space="PSUM"))
        dram_p = ctx.enter_context(tc.tile_pool(name="dram_p", bufs=2, space="DRAM"))

        # ---- input DMAs, ordered so the first projection MM starts ASAP ----
        wk_s = const.tile([P, KC, GD], F32R)
        nc.sync.dma_start(out=wk_s, in_=wk.rearrange("(kc p) f -> p kc f", p=P))
        bq_s = const.tile([P, 2], F32)
        nc.sync.dma_start(out=bq_s, in_=bq.rearrange("(fc p) -> p fc", p=P))
        xT_s = big.tile([P, KC, N], F32R)  # x^T as [128, kc, tok]
        xT_r = xT.rearrange("(kc p) n -> p kc n", p=P)
        nc.sync.dma_start(out=xT_s[:, :, 0:QC], in_=xT_r[:, :, 0:QC])

        def et_dma(qc, kt):
            # exp(bias)^T tile [128 keys, 512 q], split in 2 for queue overlap
            bt = et_pool.tile([P, QC], BF16, tag="et", name="et")
            for j in range(2):
                hw = QC // 2
                nc.sync.dma_start(
                    out=bt[:, j * hw : (j + 1) * hw],
                    in_=ebT[
                        kt * P : (kt + 1) * P,
                        qc * QC + j * hw : qc * QC + (j + 1) * hw,
                    ],
                )
            return bt

        bts = {}
        for kt in range(3):
            bts[kt] = et_dma(0, kt)

        wv_s = const.tile([P, KC, GD], F32R)
        nc.sync.dma_start(out=wv_s, in_=wv.rearrange("(kc p) f -> p kc f", p=P))
        wq_s = const.tile([P, KC, GD], F32R)
        nc.sync.dma_start(out=wq_s, in_=wq.rearrange("(kc p) f -> p kc f", p=P))
        for c in range(1, NQC):
            csl = slice(c * QC, (c + 1) * QC)
            nc.sync.dma_start(out=xT_s[:, :, csl], in_=xT_r[:, :, csl])
        # wo rows packed 2 heads per 128: wo2[p, j, :] = wo[j*128 + p, :]
        wo2_s = const.tile([P, 2, D], F32R)
        nc.sync.dma_start(out=wo2_s, in_=wo.rearrange("(j p) d -> p j d", p=P))

        bqs = const.tile([P, 2], F32)  # bq * 0.125 (scale folded into Q)
        nc.vector.tensor_scalar_mul(bqs, bq_s, 0.125)

        # Q^T, K^T: [128, fc, tok]; head h lives at partitions (h%2)*64 of
        # chunk fc=h//2 (so head pair j=(2j,2j+1) occupies all of fc=j).
        qT = big.tile([P, 2, N], F32R)
        kT = big.tile([P, 2, N], F32R)
        # V natural layout, bf16, augmented ones column: vaug[128tok, h, kt, 65]
        vaug = big.tile([P, HG, NKT, DH + 1], BF16)
        nc.vector.memset(vaug[:, :, :, DH : DH + 1], 1.0)
        # A^T 2-head-packed: aT2[p, j, q]; partitions 0:64 = head 2j,
        # 64:128 = head 2j+1 (matches wo2_s packing).
        aT2 = big.tile([P, 2, N], F32R)

        def proj_chunk(c):
            # K, V, Q projections for token chunk c
            csl = slice(c * QC, (c + 1) * QC)
            for fc in range(2):
                ps = psum_s.tile([P, QC], F32, tag="s")
                for kc in range(KC):
                    nc.tensor.matmul(
                        ps,
                        wk_s[:, kc, fc * P : (fc + 1) * P],
                        xT_s[:, kc, csl],
                        start=(kc == 0),
                        stop=(kc == KC - 1),
                    )
                nc.vector.tensor_copy(kT[:, fc, csl], ps)
            for kt in range(4 * c, 4 * c + 4):
                ps = psum_s.tile([P, GD], F32, tag="s")
                for kc in range(KC):
                    nc.tensor.matmul(
                        ps,
                        xT_s[:, kc, kt * P : (kt + 1) * P],
                        wv_s[:, kc, :],
                        start=(kc == 0),
                        stop=(kc == KC - 1),
                    )
                nc.vector.tensor_copy(
                    vaug[:, :, kt, 0:DH],
                    ps.rearrange("p (h d) -> p h d", h=HG),
                )
            for fc in range(2):
                ps = psum_s.tile([P, QC], F32, tag="s")
                for kc in range(KC):
                    nc.tensor.matmul(
                        ps,
                        wq_s[:, kc, fc * P : (fc + 1) * P],
                        xT_s[:, kc, csl],
                        start=(kc == 0),
                        stop=(kc == KC - 1),
                    )
                # (x@wq + bq) * 0.125 == psum*0.125 + bq*0.125
                nc.vector.tensor_scalar(
                    qT[:, fc, csl],
                    ps,
                    0.125,
                    bqs[:, fc : fc + 1],
                    op0=mybir.AluOpType.mult,
                    op1=mybir.AluOpType.add,
                )

        # -- deferred softmax-normalization machinery --
        def norm_part1(qc, pu):
            # evict U (frees the PSUM accumulators fast), push the r rows
            # (denominators) to DRAM and gather them onto 4 partitions.
            st = {"qc": qc}
            rd = dram_p.tile([HG, QC], F32, tag="rd", name="rd")
            uts = []
            for h in range(HG):
                ut = u_pool.tile([DH + 1, QC], F32, tag=f"ut{h}", name=f"ut{h}")
                nc.vector.tensor_copy(ut, pu[h])
                nc.gpsimd.dma_start(out=rd[h : h + 1, :], in_=ut[DH : DH + 1, :])
                uts.append(ut)
            rr = r_pool.tile([HG, QC], F32, tag="rr", name="rr")
            nc.gpsimd.dma_start(out=rr, in_=rd[:, :])
            st["uts"] = uts
            st["rr"] = rr
            return st

        def norm_part2a(st):
            # 1/r and broadcast across 64 partitions via DRAM roundtrip
            rc = r_pool.tile([HG, QC], F32, tag="rc", name="rc")
            nc.vector.reciprocal_approx_fast(out=rc, in_=st["rr"])
            rd2 = dram_p.tile([HG, QC], F32, tag="rd2", name="rd2")
            nc.gpsimd.dma_start(out=rd2[:, :], in_=rc)
            rb = r_pool.tile([DH, HG, QC], F32, tag="rb", name="rb")
            rap = rd2[:, :]
            nc.gpsimd.dma_start(
                out=rb,
                in_=bass.AP(
                    tensor=rap.tensor, offset=rap.offset,
                    ap=[[0, DH]] + list(rap.ap),
                ),
            )
            st["rb"] = rb

        def norm_part2b(st):
            # A^T = U^T * (1/r), written 2-head-packed
            qc = st["qc"]
            qsl = slice(qc * QC, (qc + 1) * QC)
            for h in range(HG):
                po = (h % 2) * DH
                nc.vector.tensor_tensor(
                    aT2[po : po + DH, h // 2, qsl],
                    st["uts"][h][0:DH, :],
                    st["rb"][:, h, :],
                    op=mybir.AluOpType.mult,
                )

        def oproj_tile(t, evict_on_act):
            # O[tok, 512] = sum_j aT2[:, j, tsl].T @ wo2[:, j, :]
            tsl = slice(t * P, (t + 1) * P)
            po = psum_s.tile([P, D], F32, tag="s", name="po")
            for j in range(2):
                nc.tensor.matmul(
                    po, aT2[:, j, tsl], wo2_s[:, j, :],
                    start=(j == 0), stop=(j == 1),
                )
            ob = o_pool.tile([P, D], F32, name="ob")
            if evict_on_act:
                nc.scalar.activation(ob, po, mybir.ActivationFunctionType.Copy)
            else:
                nc.vector.tensor_copy(ob, po)
            (nc.sync if t % 2 == 0 else nc.gpsimd).dma_start(
                out=out[tsl, :], in_=ob
            )

        # ---- main schedule ----
        proj_chunk(0)
        norm_st = None
        for qc in range(NQC):
            qsl = slice(qc * QC, (qc + 1) * QC)
            pu = [
                psum_u.tile([DH + 1, QC], F32, tag=f"u{h}", name=f"pu{h}")
                for h in range(HG)
            ]
            spb_hold = [None, None]
            for kt in range(NKT):
                if kt + 2 < NKT:
                    if (qc, kt + 2) != (0, 2):  # (0,0..2) already prefetched
                        bts[kt + 2] = et_dma(qc, kt + 2)
                else:
                    if qc + 1 < NQC:
                        bts[kt + 2 - NKT] = et_dma(qc + 1, kt + 2 - NKT)
                bt = bts.pop(kt)
                # interleave remaining projections into chunk 0 (PE slack;
                # also keeps HAM from re-throttling during the cold start)
                if qc == 0 and kt in (3, 7, 11):
                    proj_chunk(kt // 4 + 1)
                # deferred normalization of the previous chunk
                if norm_st is not None:
                    if kt == 3:
                        norm_part2a(norm_st)
                    elif kt == 8:
                        norm_part2b(norm_st)
                        norm_st = None
                for j in range(2):  # head pair j = heads (2j, 2j+1)
                    ps2 = psum_s.tile([P, 2 * QC], F32, tag="s", name="ps2")
                    for i in range(2):
                        ho = i * DH
                        nc.tensor.matmul(
                            ps2[:, i * QC : (i + 1) * QC],
                            kT[ho : ho + DH, j, kt * P : (kt + 1) * P],
                            qT[ho : ho + DH, j, qsl],
                            start=True,
                            stop=True,
                        )
                    # PE: PV for the PREVIOUS kt right after this pair's
                    # score MMs — exp/mult of kt-1 are long done by now.
                    if kt > 0:
                        sprev = spb_hold[j]
                        for i in range(2):
                            nc.tensor.matmul(
                                pu[2 * j + i],
                                vaug[:, 2 * j + i, kt - 1, :],
                                sprev[:, i * QC : (i + 1) * QC],
                                start=(kt - 1 == 0),
                                stop=False,
                            )
                    sp = sp_pool.tile([P, 2 * QC], BF16, tag="sp", name="sp")
                    nc.scalar.activation(sp, ps2, mybir.ActivationFunctionType.Exp)
                    spb = spb_pool.tile([P, 2 * QC], BF16, tag="spb", name="spb")
                    for i in range(2):
                        isl = slice(i * QC, (i + 1) * QC)
                        nc.vector.tensor_tensor(
                            spb[:, isl], sp[:, isl], bt, op=mybir.AluOpType.mult
                        )
                    spb_hold[j] = spb
            for j in range(2):  # PV for kt = NKT-1
                sprev = spb_hold[j]
                for i in range(2):
                    nc.tensor.matmul(
                        pu[2 * j + i],
                        vaug[:, 2 * j + i, NKT - 1, :],
                        sprev[:, i * QC : (i + 1) * QC],
                        start=False,
                        stop=True,
                    )
            if qc == NQC - 1:
                # keep the PE hot with O-proj of chunks 0-2 while the last
                # normalization chain runs on DVE/DMA
                for t in range(12):
                    oproj_tile(t, evict_on_act=(t % 2 == 1))
            norm_st = norm_part1(qc, pu)
        norm_part2a(norm_st)
        norm_part2b(norm_st)
        for t in range(12, 16):
            oproj_tile(t, evict_on_act=(t % 2 == 1))

    nc.compile()
    return nc


_NC = None


def _get_nc():
    global _NC
    if _NC is None:
        _NC = build_program()
    return _NC


def make_in_maps(x, attn_bias, wq, bq, wk, bk, wv, bv, wo, bo):
    x = np.asarray(x, np.float32)
    attn_bias = np.asarray(attn_bias, np.float32)
    # exp(bias)^T per batch, bf16 (shared by the 2 cores of each batch)
    ebTs = [
        np.exp(attn_bias[b, 0].T).astype(ml_dtypes.bfloat16) for b in range(B)
    ]
    xTs = [np.ascontiguousarray(x[b].T) for b in range(B)]
    in_maps = []
    for c in range(8):
        b, g = c // 2, c % 2
        sl = slice(g * GD, (g + 1) * GD)
        in_maps.append(
            {
                "xT": xTs[b],
                "ebT": ebTs[b],
                "wq": np.ascontiguousarray(np.asarray(wq, np.float32)[:, sl]),
                "wk": np.ascontiguousarray(np.asarray(wk, np.float32)[:, sl]),
                "wv": np.ascontiguousarray(np.asarray(wv, np.float32)[:, sl]),
                "wo": np.ascontiguousarray(np.asarray(wo, np.float32)[sl, :]),
                "bq": np.ascontiguousarray(np.asarray(bq, np.float32)[sl]),
            }
        )
    return in_maps


def gather_output(results, bo, bv, wo):
    bo = np.asarray(bo, np.float32)
    row = bo + np.asarray(bv, np.float32) @ np.asarray(wo, np.float32)
    out = np.empty((B, N, D), np.float32)
    for b in range(B):
        out[b] = results[2 * b]["out"] + results[2 * b + 1]["out"] + row[None, :]
    return out


def kernel(x, attn_bias, wq, bq, wk, bk, wv, bv, wo, bo, _trace=False):
    nc = _get_nc()
    in_maps = make_in_maps(x, attn_bias, wq, bq, wk, bk, wv, bv, wo, bo)
    res = run_bass_kernel_spmd(nc, in_maps, core_ids=list(range(8)), trace=_trace)
    out = gather_output(res.results, bo, bv, wo)
    if _trace:
        kernel.last_results = res
    return out


# revision 4
# speedup vs baseline: 1.6987x; 1.0427x over previous
"""Biased MHSA Trainium2 kernel (8-core SPMD), v4.

Sharding: core c -> (batch b = c//2, head-group g = c%2); each core computes
attention for 4 of the 8 heads of one batch and the partial output projection
for those heads. Host sums the two head-group partials per batch and adds
bo + bv @ wo (bv folded via softmax row-sum = 1; bk dropped entirely since a
per-query constant shift cancels in softmax).

Key structure:
  - exp(S + bias) = exp(S) * exp(bias): exp(bias) precomputed on HOST, bf16.
  - One ACT exp per head-pair on a [128,1024] scores PSUM tile (double
    buffered); DVE multiplies by exp(bias) in bf16 2x-packed mode.
  - PV accumulates into 4 single-bank [65,512] PSUM tiles (bf16 V + ones
    column = softmax denominator). PE emission interleaves PV of kt-1 after
    the score MMs of kt so the PE never waits on the exp->mult chain.
  - Projections for token chunks 1-3 are interleaved into attention chunk
    0's key loop; O-proj tiles of chunk qc-1 are interleaved into chunk
    qc's key loop (kt 9/11/13/15). Both keep the PE dense so the HAM
    activity monitor never re-throttles the PE clock to half rate.
  - Softmax normalization is split: at the chunk boundary only the PSUM
    eviction + r-row DMAs are emitted (frees the accumulators); the
    reciprocal / broadcast / A^T writes are deferred into the next chunk's
    key loop, emitted AFTER that kt's softmax multiplies so the in-order
    DVE queue never delays the PV operands.
  - Startup DMAs are spread across the Sync and Scalar hardware DGE queues
    (ACT is idle early) so the first projection matmul starts ~12us in.
"""

import sys

if "/opt/trn_rl_repo" not in sys.path:
    sys.path.insert(0, "/opt/trn_rl_repo")

from contextlib import ExitStack

import numpy as np
import ml_dtypes

import concourse.bass as bass
from concourse import bacc
import concourse.tile as tile
from concourse import mybir
from concourse.bass_utils import run_bass_kernel_spmd

B, N, D = 4, 2048, 512
H, DH = 8, 64
HG = 4  # heads per core
GD = HG * DH  # 256 features per core
P = 128
QC = 512  # q processed in chunks of 512
NQC = N // QC  # 4 q chunks
NKT = N // P  # 16 key tiles
KC = D // P  # 4 contraction chunks for projections
F32 = mybir.dt.float32
F32R = mybir.dt.float32r
BF16 = mybir.dt.bfloat16


def build_program():
    nc = bacc.Bacc("TRN2", target_bir_lowering=False)
    xT = nc.dram_tensor("xT", [D, N], F32R, kind="ExternalInput")
    ebT = nc.dram_tensor("ebT", [N, N], BF16, kind="ExternalInput")  # exp(bias)^T
    wq = nc.dram_tensor("wq", [D, GD], F32R, kind="ExternalInput")
    wk = nc.dram_tensor("wk", [D, GD], F32R, kind="ExternalInput")
    wv = nc.dram_tensor("wv", [D, GD], F32R, kind="ExternalInput")
    wo = nc.dram_tensor("wo", [GD, D], F32R, kind="ExternalInput")
    bq = nc.dram_tensor("bq", [GD], F32, kind="ExternalInput")
    out = nc.dram_tensor("out", [N, D], F32, kind="ExternalOutput")

    with tile.TileContext(nc) as tc, ExitStack() as ctx:
        const = ctx.enter_context(tc.tile_pool(name="const", bufs=1))
        big = ctx.enter_context(tc.tile_pool(name="big", bufs=1))
        et_pool = ctx.enter_context(tc.tile_pool(name="etp", bufs=4))
        sp_pool = ctx.enter_context(tc.tile_pool(name="spp", bufs=3))
        spb_pool = ctx.enter_context(tc.tile_pool(name="spbp", bufs=3))
        u_pool = ctx.enter_context(tc.tile_pool(name="up", bufs=2))
        r_pool = ctx.enter_context(tc.tile_pool(name="rp", bufs=2))
        o_pool = ctx.enter_context(tc.tile_pool(name="op", bufs=3))
        psum_s = ctx.enter_context(tc.tile_pool(name="psum_s", bufs=2, space="PSUM"))
        psum_u = ctx.enter_context(tc.tile_pool(name="psum_u", bufs=1, space="PSUM"))
        dram_p = ctx.enter_context(tc.tile_pool(name="dram_p", bufs=2, space="DRAM"))

        # ---- input DMAs: sync queue carries xT c0/c1 + the et stream;
        # scalar (ACT) hw queue carries weights + xT c2/c3 (ACT idle early)
        xT_s = big.tile([P, KC, N], F32R)  # x^T as [128, kc, tok]
        xT_r = xT.rearrange("(kc p) n -> p kc n", p=P)
        nc.sync.dma_start(out=xT_s[:, :, 0:QC], in_=xT_r[:, :, 0:QC])
        nc.sync.dma_start(out=xT_s[:, :, QC : 2 * QC], in_=xT_r[:, :, QC : 2 * QC])

        wk_s = const.tile([P, KC, GD], F32R)
        nc.scalar.dma_start(out=wk_s, in_=wk.rearrange("(kc p) f -> p kc f", p=P))
        bq_s = const.tile([P, 2], F32)
        nc.scalar.dma_start(out=bq_s, in_=bq.rearrange("(fc p) -> p fc", p=P))
        wv_s = const.tile([P, KC, GD], F32R)
        nc.scalar.dma_start(out=wv_s, in_=wv.rearrange("(kc p) f -> p kc f", p=P))
        wq_s = const.tile([P, KC, GD], F32R)
        nc.scalar.dma_start(out=wq_s, in_=wq.rearrange("(kc p) f -> p kc f", p=P))
        for c in range(2, NQC):
            csl = slice(c * QC, (c + 1) * QC)
            nc.scalar.dma_start(out=xT_s[:, :, csl], in_=xT_r[:, :, csl])
        # wo rows packed 2 heads per 128: wo2[p, j, :] = wo[j*128 + p, :]
        wo2_s = const.tile([P, 2, D], F32R)
        nc.scalar.dma_start(out=wo2_s, in_=wo.rearrange("(j p) d -> p j d", p=P))

        def et_dma(qc, kt):
            # exp(bias)^T tile [128 keys, 512 q]
            bt = et_pool.tile([P, QC], BF16, tag="et", name="et")
            nc.sync.dma_start(
                out=bt, in_=ebT[kt * P : (kt + 1) * P, qc * QC : (qc + 1) * QC]
            )
            return bt

        bts = {}
        for kt in range(3):
            bts[kt] = et_dma(0, kt)

        bqs = const.tile([P, 2], F32)  # bq * 0.125 (scale folded into Q)
        nc.vector.tensor_scalar_mul(bqs, bq_s, 0.125)

        # Q^T, K^T: [128, fc, tok]; head h lives at partitions (h%2)*64 of
        # chunk fc=h//2 (so head pair j=(2j,2j+1) occupies all of fc=j).
        qT = big.tile([P, 2, N], F32R)
        kT = big.tile([P, 2, N], F32R)
        # V natural layout, bf16, augmented ones column: vaug[128tok, h, kt, 65]
        vaug = big.tile([P, HG, NKT, DH + 1], BF16)
        nc.vector.memset(vaug[:, :, :, DH : DH + 1], 1.0)
        # A^T 2-head-packed: aT2[p, j, q]; partitions 0:64 = head 2j,
        # 64:128 = head 2j+1 (matches wo2_s packing).
        aT2 = big.tile([P, 2, N], F32R)

        def proj_chunk(c):
            # K, V, Q projections for token chunk c
            csl = slice(c * QC, (c + 1) * QC)
            for fc in range(2):
                ps = psum_s.tile([P, QC], F32, tag="s")
                for kc in range(KC):
                    nc.tensor.matmul(
                        ps,
                        wk_s[:, kc, fc * P : (fc + 1) * P],
                        xT_s[:, kc, csl],
                        start=(kc == 0),
                        stop=(kc == KC - 1),
                    )
                nc.vector.tensor_copy(kT[:, fc, csl], ps)
            for kt in range(4 * c, 4 * c + 4):
                ps = psum_s.tile([P, GD], F32, tag="s")
                for kc in range(KC):
                    nc.tensor.matmul(
                        ps,
                        xT_s[:, kc, kt * P : (kt + 1) * P],
                        wv_s[:, kc, :],
                        start=(kc == 0),
                        stop=(kc == KC - 1),
                    )
                nc.vector.tensor_copy(
                    vaug[:, :, kt, 0:DH],
                    ps.rearrange("p (h d) -> p h d", h=HG),
                )
            for fc in range(2):
                ps = psum_s.tile([P, QC], F32, tag="s")
                for kc in range(KC):
                    nc.tensor.matmul(
                        ps,
                        wq_s[:, kc, fc * P : (fc + 1) * P],
                        xT_s[:, kc, csl],
                        start=(kc == 0),
                        stop=(kc == KC - 1),
                    )
                # (x@wq + bq) * 0.125 == psum*0.125 + bq*0.125
                nc.vector.tensor_scalar(
                    qT[:, fc, csl],
                    ps,
                    0.125,
                    bqs[:, fc : fc + 1],
                    op0=mybir.AluOpType.mult,
                    op1=mybir.AluOpType.add,
                )

        # -- deferred softmax-normalization machinery --
        def norm_part1(qc, pu):
            # evict U (frees the PSUM accumulators fast), push the r rows
            # (denominators) to DRAM and gather them onto 4 partitions.
            st = {"qc": qc}
            rd = dram_p.tile([HG, QC], F32, tag="rd", name="rd")
            uts = []
            for h in range(HG):
                ut = u_pool.tile([DH + 1, QC], F32, tag=f"ut{h}", name=f"ut{h}")
                nc.vector.tensor_copy(ut, pu[h])
                nc.gpsimd.dma_start(out=rd[h : h + 1, :], in_=ut[DH : DH + 1, :])
                uts.append(ut)
            rr = r_pool.tile([HG, QC], F32, tag="rr", name="rr")
            nc.gpsimd.dma_start(out=rr, in_=rd[:, :])
            st["uts"] = uts
            st["rr"] = rr
            return st

        def norm_part2a(st):
            # 1/r and broadcast across 64 partitions via DRAM roundtrip
            rc = r_pool.tile([HG, QC], F32, tag="rc", name="rc")
            nc.vector.reciprocal_approx_fast(out=rc, in_=st["rr"])
            rd2 = dram_p.tile([HG, QC], F32, tag="rd2", name="rd2")
            nc.gpsimd.dma_start(out=rd2[:, :], in_=rc)
            rb = r_pool.tile([DH, HG, QC], F32, tag="rb", name="rb")
            rap = rd2[:, :]
            nc.gpsimd.dma_start(
                out=rb,
                in_=bass.AP(
                    tensor=rap.tensor, offset=rap.offset,
                    ap=[[0, DH]] + list(rap.ap),
                ),
            )
            st["rb"] = rb

        def norm_part2b(st, hs):
            # A^T = U^T * (1/r), written 2-head-packed
            qc = st["qc"]
            qsl = slice(qc * QC, (qc + 1) * QC)
            for h in hs:
                po = (h % 2) * DH
                nc.vector.tensor_tensor(
                    aT2[po : po + DH, h // 2, qsl],
                    st["uts"][h][0:DH, :],
                    st["rb"][:, h, :],
                    op=mybir.AluOpType.mult,
                )

        def oproj_tile(t, evict_on_act=False, dma_on_scalar=False):
            # O[tok, 512] = sum_j aT2[:, j, tsl].T @ wo2[:, j, :]
            tsl = slice(t * P, (t + 1) * P)
            po = psum_s.tile([P, D], F32, tag="s", name="po")
            for j in range(2):
                nc.tensor.matmul(
                    po, aT2[:, j, tsl], wo2_s[:, j, :],
                    start=(j == 0), stop=(j == 1),
                )
            ob = o_pool.tile([P, D], F32, name="ob")
            if evict_on_act:
                nc.scalar.activation(ob, po, mybir.ActivationFunctionType.Copy)
            else:
                nc.vector.tensor_copy(ob, po)
            (nc.scalar if dma_on_scalar else nc.sync).dma_start(
                out=out[tsl, :], in_=ob
            )

        # ---- main schedule ----
        proj_chunk(0)
        norm_st = None
        for qc in range(NQC):
            qsl = slice(qc * QC, (qc + 1) * QC)
            pu = [
                psum_u.tile([DH + 1, QC], F32, tag=f"u{h}", name=f"pu{h}")
                for h in range(HG)
            ]
            spb_hold = [None, None]
            for kt in range(NKT):
                if kt + 2 < NKT:
                    if (qc, kt + 2) != (0, 2):  # (0,0..2) already prefetched
                        bts[kt + 2] = et_dma(qc, kt + 2)
                else:
                    if qc + 1 < NQC:
                        bts[kt + 2 - NKT] = et_dma(qc + 1, kt + 2 - NKT)
                bt = bts.pop(kt)
                # interleave remaining projections into chunk 0 (PE slack;
                # also keeps HAM from re-throttling during the cold start)
                if qc == 0 and kt in (3, 7, 11):
                    proj_chunk(kt // 4 + 1)
                for j in range(2):  # head pair j = heads (2j, 2j+1)
                    ps2 = psum_s.tile([P, 2 * QC], F32, tag="s", name="ps2")
                    for i in range(2):
                        ho = i * DH
                        nc.tensor.matmul(
                            ps2[:, i * QC : (i + 1) * QC],
                            kT[ho : ho + DH, j, kt * P : (kt + 1) * P],
                            qT[ho : ho + DH, j, qsl],
                            start=True,
                            stop=True,
                        )
                    # PE: PV for the PREVIOUS kt right after this pair's
                    # score MMs — exp/mult of kt-1 are long done by now.
                    if kt > 0:
                        sprev = spb_hold[j]
                        for i in range(2):
                            nc.tensor.matmul(
                                pu[2 * j + i],
                                vaug[:, 2 * j + i, kt - 1, :],
                                sprev[:, i * QC : (i + 1) * QC],
                                start=(kt - 1 == 0),
                                stop=False,
                            )
                    sp = sp_pool.tile([P, 2 * QC], BF16, tag="sp", name="sp")
                    nc.scalar.activation(sp, ps2, mybir.ActivationFunctionType.Exp)
                    spb = spb_pool.tile([P, 2 * QC], BF16, tag="spb", name="spb")
                    for i in range(2):
                        isl = slice(i * QC, (i + 1) * QC)
                        nc.vector.tensor_tensor(
                            spb[:, isl], sp[:, isl], bt, op=mybir.AluOpType.mult
                        )
                    spb_hold[j] = spb
                # deferred work, emitted AFTER this kt's critical DVE mults
                # so the in-order DVE queue never delays the PV operands
                if norm_st is not None:
                    if kt == 4:
                        norm_part2a(norm_st)
                    elif kt == 8:
                        norm_part2b(norm_st, (0, 1))
                    elif kt == 9:
                        norm_part2b(norm_st, (2, 3))
                        norm_st = None
                # O-proj of chunk qc-1 interleaved once aT2(qc-1) is ready;
                # fills the ACT-vs-PE pace gap to keep the PE HAM-dense.
                if qc >= 1 and kt in (9, 11, 13, 15):
                    oproj_tile(4 * (qc - 1) + (kt - 9) // 2)
            for j in range(2):  # PV for kt = NKT-1
                sprev = spb_hold[j]
                for i in range(2):
                    nc.tensor.matmul(
                        pu[2 * j + i],
                        vaug[:, 2 * j + i, NKT - 1, :],
                        sprev[:, i * QC : (i + 1) * QC],
                        start=False,
                        stop=True,
                    )
            norm_st = norm_part1(qc, pu)
        norm_part2a(norm_st)
        norm_part2b(norm_st, (0, 1, 2, 3))
        for t in range(12, 16):
            oproj_tile(t, evict_on_act=(t % 2 == 1), dma_on_scalar=(t % 2 == 1))

    nc.compile()
    return nc


_NC = None


def _get_nc():
    global _NC
    if _NC is None:
        _NC = build_program()
    return _NC


def make_in_maps(x, attn_bias, wq, bq, wk, bk, wv, bv, wo, bo):
    x = np.asarray(x, np.float32)
    attn_bias = np.asarray(attn_bias, np.float32)
    # exp(bias)^T per batch, bf16 (shared by the 2 cores of each batch)
    ebTs = [
        np.exp(attn_bias[b, 0].T).astype(ml_dtypes.bfloat16) for b in range(B)
    ]
    xTs = [np.ascontiguousarray(x[b].T) for b in range(B)]
    in_maps = []
    for c in range(8):
        b, g = c // 2, c % 2
        sl = slice(g * GD, (g + 1) * GD)
        in_maps.append(
            {
                "xT": xTs[b],
                "ebT": ebTs[b],
                "wq": np.ascontiguousarray(np.asarray(wq, np.float32)[:, sl]),
                "wk": np.ascontiguousarray(np.asarray(wk, np.float32)[:, sl]),
                "wv": np.ascontiguousarray(np.asarray(wv, np.float32)[:, sl]),
                "wo": np.ascontiguousarray(np.asarray(wo, np.float32)[sl, :]),
                "bq": np.ascontiguousarray(np.asarray(bq, np.float32)[sl]),
            }
        )
    return in_maps


def gather_output(results, bo, bv, wo):
    bo = np.asarray(bo, np.float32)
    row = bo + np.asarray(bv, np.float32) @ np.asarray(wo, np.float32)
    out = np.empty((B, N, D), np.float32)
    for b in range(B):
        out[b] = results[2 * b]["out"] + results[2 * b + 1]["out"] + row[None, :]
    return out


def kernel(x, attn_bias, wq, bq, wk, bk, wv, bv, wo, bo, _trace=False):
    nc = _get_nc()
    in_maps = make_in_maps(x, attn_bias, wq, bq, wk, bk, wv, bv, wo, bo)
    res = run_bass_kernel_spmd(nc, in_maps, core_ids=list(range(8)), trace=_trace)
    out = gather_output(res.results, bo, bv, wo)
    if _trace:
        kernel.last_results = res
    return out


# revision 10
# speedup vs baseline: 1.7993x; 1.0592x over previous
"""Biased MHSA Trainium2 kernel (8-core SPMD), v5.

Sharding: core c -> (batch b = c//2, head-group g = c%2); each core computes
attention for 4 of the 8 heads of one batch and the partial output projection
for those heads. Host sums the two head-group partials per batch and adds
bo + bv @ wo (bv folded via softmax row-sum = 1; bk dropped entirely since a
per-query constant shift cancels in softmax).

Key structure:
  - exp(S + bias) = exp(S) * exp(bias): exp(bias) precomputed on HOST, bf16.
  - x and wq/wk/wv shipped bf16 (projection matmuls bf16, fp32 PSUM).
  - One ACT exp per head-pair on a [128,1024] scores PSUM tile (double
    buffered); DVE multiplies by exp(bias) in bf16 2x-packed mode.
  - PV accumulates into 4 single-bank [65,512] PSUM tiles (bf16 V + ones
    column = softmax denominator). PE emission interleaves PV of kt-1 after
    the score MMs of kt so the PE never waits on the exp->mult chain.
  - Projections for token chunks 1-3 are interleaved into attention chunk
    0's key loop; O-proj tiles of chunk qc-1 are interleaved into chunk
    qc's key loop. Both keep the PE dense so the HAM activity monitor never
    re-throttles the PE clock to half rate.
  - Softmax normalization is pipelined across chunks with at most ONE
    deferred DVE op per key-tile (the in-order DVE queue must never delay
    the PV operands): boundary = U eviction + r-row DMAs + gather; kt3 =
    reciprocal; kt8-11 = one A^T write per kt; kt12-15 = one O-proj tile
    per kt. The last chunk instead broadcasts 1/r through the PE (ones
    matmul into the freed PSUM accumulator banks) to avoid the slow DMA
    broadcast on the critical tail.
"""

import sys

if "/opt/trn_rl_repo" not in sys.path:
    sys.path.insert(0, "/opt/trn_rl_repo")

from contextlib import ExitStack

import numpy as np
import ml_dtypes

import concourse.bass as bass
from concourse import bacc
import concourse.tile as tile
from concourse import mybir
from concourse.bass_utils import run_bass_kernel_spmd

B, N, D = 4, 2048, 512
H, DH = 8, 64
HG = 4  # heads per core
GD = HG * DH  # 256 features per core
P = 128
QC = 512  # q processed in chunks of 512
NQC = N // QC  # 4 q chunks
NKT = N // P  # 16 key tiles
KC = D // P  # 4 contraction chunks for projections
F32 = mybir.dt.float32
F32R = mybir.dt.float32r
BF16 = mybir.dt.bfloat16
BF16NP = ml_dtypes.bfloat16


def build_program():
    nc = bacc.Bacc("TRN2", target_bir_lowering=False)
    xT = nc.dram_tensor("xT", [D, N], BF16, kind="ExternalInput")
    ebT = nc.dram_tensor("ebT", [N, N], BF16, kind="ExternalInput")  # exp(bias)^T
    wq = nc.dram_tensor("wq", [D, GD], BF16, kind="ExternalInput")
    wk = nc.dram_tensor("wk", [D, GD], BF16, kind="ExternalInput")
    wv = nc.dram_tensor("wv", [D, GD], BF16, kind="ExternalInput")
    wo = nc.dram_tensor("wo", [GD, D], F32R, kind="ExternalInput")
    bq = nc.dram_tensor("bq", [GD], F32, kind="ExternalInput")
    out = nc.dram_tensor("out", [N, D], F32, kind="ExternalOutput")

    with tile.TileContext(nc) as tc, ExitStack() as ctx:
        const = ctx.enter_context(tc.tile_pool(name="const", bufs=1))
        big = ctx.enter_context(tc.tile_pool(name="big", bufs=1))
        et_pool = ctx.enter_context(tc.tile_pool(name="etp", bufs=4))
        sp_pool = ctx.enter_context(tc.tile_pool(name="spp", bufs=3))
        spb_pool = ctx.enter_context(tc.tile_pool(name="spbp", bufs=3))
        u_pool = ctx.enter_context(tc.tile_pool(name="up", bufs=2))
        r_pool = ctx.enter_context(tc.tile_pool(name="rp", bufs=2))
        o_pool = ctx.enter_context(tc.tile_pool(name="op", bufs=3))
        psum_s = ctx.enter_context(tc.tile_pool(name="psum_s", bufs=2, space="PSUM"))
        psum_u = ctx.enter_context(tc.tile_pool(name="psum_u", bufs=1, space="PSUM"))
        dram_p = ctx.enter_context(tc.tile_pool(name="dram_p", bufs=2, space="DRAM"))

        # ---- input DMAs: sync queue carries xT c0/c1 + the et stream;
        # scalar (ACT) hw queue carries weights + xT c2/c3 (ACT idle early)
        xT_s = big.tile([P, KC, N], BF16)  # x^T as [128, kc, tok]
        xT_r = xT.rearrange("(kc p) n -> p kc n", p=P)
        nc.sync.dma_start(out=xT_s[:, :, 0:QC], in_=xT_r[:, :, 0:QC])
        nc.sync.dma_start(out=xT_s[:, :, QC : 2 * QC], in_=xT_r[:, :, QC : 2 * QC])

        wk_s = const.tile([P, KC, GD], BF16)
        nc.scalar.dma_start(out=wk_s, in_=wk.rearrange("(kc p) f -> p kc f", p=P))
        bq_s = const.tile([P, 2], F32)
        nc.scalar.dma_start(out=bq_s, in_=bq.rearrange("(fc p) -> p fc", p=P))
        wv_s = const.tile([P, KC, GD], BF16)
        nc.scalar.dma_start(out=wv_s, in_=wv.rearrange("(kc p) f -> p kc f", p=P))
        wq_s = const.tile([P, KC, GD], BF16)
        nc.scalar.dma_start(out=wq_s, in_=wq.rearrange("(kc p) f -> p kc f", p=P))
        for c in range(2, NQC):
            csl = slice(c * QC, (c + 1) * QC)
            nc.scalar.dma_start(out=xT_s[:, :, csl], in_=xT_r[:, :, csl])
        # wo rows packed 2 heads per 128: wo2[p, j, :] = wo[j*128 + p, :]
        wo2_s = const.tile([P, 2, D], F32R)
        nc.scalar.dma_start(out=wo2_s, in_=wo.rearrange("(j p) d -> p j d", p=P))

        def et_dma(qc, kt):
            # exp(bias)^T tile [128 keys, 512 q]
            bt = et_pool.tile([P, QC], BF16, tag="et", name="et")
            nc.sync.dma_start(
                out=bt, in_=ebT[kt * P : (kt + 1) * P, qc * QC : (qc + 1) * QC]
            )
            return bt

        bts = {}
        for kt in range(3):
            bts[kt] = et_dma(0, kt)

        bqs = const.tile([P, 2], F32)  # bq * 0.125 (scale folded into Q)
        nc.vector.tensor_scalar_mul(bqs, bq_s, 0.125)
        ones97 = const.tile([97, DH], F32R)
        nc.vector.memset(ones97.bitcast(F32), 1.0)

        # Q^T, K^T: [128, fc, tok]; head h lives at partitions (h%2)*64 of
        # chunk fc=h//2 (so head pair j=(2j,2j+1) occupies all of fc=j).
        qT = big.tile([P, 2, N], BF16)
        kT = big.tile([P, 2, N], BF16)
        # V natural layout, bf16, augmented ones column: vaug[128tok, h, kt, 65]
        vaug = big.tile([P, HG, NKT, DH + 1], BF16)
        nc.vector.memset(vaug[:, :, :, DH : DH + 1], 1.0)
        # A^T 2-head-packed: aT2[p, j, q]; partitions 0:64 = head 2j,
        # 64:128 = head 2j+1 (matches wo2_s packing).
        aT2 = big.tile([P, 2, N], F32R)

        def proj_chunk(c):
            # K, V, Q projections for token chunk c
            csl = slice(c * QC, (c + 1) * QC)
            for fc in range(2):
                ps = psum_s.tile([P, QC], F32, tag="s")
                for kc in range(KC):
                    nc.tensor.matmul(
                        ps,
                        wk_s[:, kc, fc * P : (fc + 1) * P],
                        xT_s[:, kc, csl],
                        start=(kc == 0),
                        stop=(kc == KC - 1),
                    )
                nc.vector.tensor_copy(kT[:, fc, csl], ps)
            for kt in range(4 * c, 4 * c + 4):
                ps = psum_s.tile([P, GD], F32, tag="s")
                for kc in range(KC):
                    nc.tensor.matmul(
                        ps,
                        xT_s[:, kc, kt * P : (kt + 1) * P],
                        wv_s[:, kc, :],
                        start=(kc == 0),
                        stop=(kc == KC - 1),
                    )
                nc.vector.tensor_copy(
                    vaug[:, :, kt, 0:DH],
                    ps.rearrange("p (h d) -> p h d", h=HG),
                )
            for fc in range(2):
                ps = psum_s.tile([P, QC], F32, tag="s")
                for kc in range(KC):
                    nc.tensor.matmul(
                        ps,
                        wq_s[:, kc, fc * P : (fc + 1) * P],
                        xT_s[:, kc, csl],
                        start=(kc == 0),
                        stop=(kc == KC - 1),
                    )
                # (x@wq + bq) * 0.125 == psum*0.125 + bq*0.125
                nc.vector.tensor_scalar(
                    qT[:, fc, csl],
                    ps,
                    0.125,
                    bqs[:, fc : fc + 1],
                    op0=mybir.AluOpType.mult,
                    op1=mybir.AluOpType.add,
                )

        # -- softmax-normalization machinery, pipelined across chunks --
        def norm_part1(qc, pu, last=False):
            # evict U (frees the PSUM accumulators fast), push the r rows
            # (denominators) out and gather them onto 4 partitions.
            st = {"qc": qc}
            rd = dram_p.tile([HG, QC], F32, tag="rd", name="rd")
            uts = []
            for h in range(HG):
                ut = u_pool.tile([DH + 1, QC], F32, tag=f"ut{h}", name=f"ut{h}")
                nc.vector.tensor_copy(ut, pu[h])
                # on the tail, spread the small DMAs over idle hw queues
                eng = nc.gpsimd if not last else (nc.sync if h % 2 == 0 else nc.scalar)
                eng.dma_start(out=rd[h : h + 1, :], in_=ut[DH : DH + 1, :])
                uts.append(ut)
            rr = r_pool.tile([HG, QC], F32, tag="rr", name="rr")
            (nc.gpsimd if not last else nc.sync).dma_start(out=rr, in_=rd[:, :])
            st["uts"] = uts
            st["rr"] = rr
            return st

        def norm_recip(st):
            rc = r_pool.tile([HG, QC], F32, tag="rc", name="rc")
            nc.vector.reciprocal_approx_fast(out=rc, in_=st["rr"])
            st["rc"] = rc

        def norm_bcast_dma(st):
            # broadcast 1/r across 64 partitions via DRAM roundtrip (hidden
            # under the next chunk's key loop; gpsimd queue is idle there)
            rd2 = dram_p.tile([HG, QC], F32, tag="rd2", name="rd2")
            nc.gpsimd.dma_start(out=rd2[:, :], in_=st["rc"])
            rb = r_pool.tile([DH, HG, QC], F32, tag="rb", name="rb")
            rap = rd2[:, :]
            nc.gpsimd.dma_start(
                out=rb,
                in_=bass.AP(
                    tensor=rap.tensor, offset=rap.offset,
                    ap=[[0, DH]] + list(rap.ap),
                ),
            )
            st["rb_ap"] = [rb[:, h, :] for h in range(HG)]

        def norm_bcast_pe(st):
            # tail path: broadcast 1/r through the PE into the freed PSUM
            # accumulator banks — no slow DMA broadcast on the critical tail
            # matmul operands may only start at partition 0/32/64: heads
            # 0-2 live at rows 0/32/64 of rc97, head 3 in its own tile
            rc97 = r_pool.tile([65, QC], F32R, tag="rc97", name="rc97")
            rcx = r_pool.tile([1, QC], F32R, tag="rcx", name="rcx")
            for h in range(HG):
                dst = rc97[32 * h : 32 * h + 1, :] if h < 3 else rcx[0:1, :]
                (nc.sync if h % 2 == 0 else nc.scalar).dma_start(
                    out=dst.bitcast(F32), in_=st["rc"][h : h + 1, :]
                )
            aps = []
            for h in range(HG):
                src = rc97[32 * h : 32 * h + 1, :] if h < 3 else rcx[0:1, :]
                lhs = ones97[32 * h : 32 * h + 1, :] if h < 3 else ones97[0:1, :]
                rbp = psum_u.tile([DH, QC], F32, tag=f"u{h}", name=f"rbp{h}")
                nc.tensor.matmul(rbp, lhs, src, start=True, stop=True)
                aps.append(rbp[:, :])
            st["rb_ap"] = aps

        def norm_aT2(st, h):
            # A^T = U^T * (1/r), written 2-head-packed
            qc = st["qc"]
            qsl = slice(qc * QC, (qc + 1) * QC)
            po = (h % 2) * DH
            nc.vector.tensor_tensor(
                aT2[po : po + DH, h // 2, qsl],
                st["uts"][h][0:DH, :],
                st["rb_ap"][h],
                op=mybir.AluOpType.mult,
            )

        def oproj_tile(t, evict_on_act=False, dma_on_scalar=False):
            # O[tok, 512] = sum_j aT2[:, j, tsl].T @ wo2[:, j, :]
            tsl = slice(t * P, (t + 1) * P)
            po = psum_s.tile([P, D], F32, tag="s", name="po")
            for j in range(2):
                nc.tensor.matmul(
                    po, aT2[:, j, tsl], wo2_s[:, j, :],
                    start=(j == 0), stop=(j == 1),
                )
            ob = o_pool.tile([P, D], F32, name="ob")
            if evict_on_act:
                nc.scalar.activation(ob, po, mybir.ActivationFunctionType.Copy)
            else:
                nc.vector.tensor_copy(ob, po)
            (nc.scalar if dma_on_scalar else nc.sync).dma_start(
                out=out[tsl, :], in_=ob
            )

        # ---- main schedule ----
        proj_chunk(0)
        norm_st = None
        for qc in range(NQC):
            qsl = slice(qc * QC, (qc + 1) * QC)
            pu = [
                psum_u.tile([DH + 1, QC], F32, tag=f"u{h}", name=f"pu{h}")
                for h in range(HG)
            ]
            spb_hold = [None, None]
            for kt in range(NKT):
                if kt + 2 < NKT:
                    if (qc, kt + 2) != (0, 2):  # (0,0..2) already prefetched
                        bts[kt + 2] = et_dma(qc, kt + 2)
                else:
                    if qc + 1 < NQC:
                        bts[kt + 2 - NKT] = et_dma(qc + 1, kt + 2 - NKT)
                bt = bts.pop(kt)
                # interleave remaining projections into chunk 0 (PE slack;
                # also keeps HAM from re-throttling during the cold start)
                if qc == 0 and kt in (3, 7, 11):
                    proj_chunk(kt // 4 + 1)
                # at kt==15 emit head-pair 1 first: the next chunk's first
                # score MM then WAR-waits on the EARLIER exp, not the later
                for j in (0, 1) if kt < NKT - 1 else (1, 0):
                    ps2 = psum_s.tile([P, 2 * QC], F32, tag="s", name="ps2")
                    for i in range(2):
                        ho = i * DH
                        nc.tensor.matmul(
                            ps2[:, i * QC : (i + 1) * QC],
                            kT[ho : ho + DH, j, kt * P : (kt + 1) * P],
                            qT[ho : ho + DH, j, qsl],
                            start=True,
                            stop=True,
                        )
                    # PE: PV for the PREVIOUS kt right after this pair's
                    # score MMs — exp/mult of kt-1 are long done by now.
                    if kt > 0:
                        sprev = spb_hold[j]
                        for i in range(2):
                            nc.tensor.matmul(
                                pu[2 * j + i],
                                vaug[:, 2 * j + i, kt - 1, :],
                                sprev[:, i * QC : (i + 1) * QC],
                                start=(kt - 1 == 0),
                                stop=False,
                            )
                    sp = sp_pool.tile([P, 2 * QC], BF16, tag="sp", name="sp")
                    nc.scalar.activation(sp, ps2, mybir.ActivationFunctionType.Exp)
                    spb = spb_pool.tile([P, 2 * QC], BF16, tag="spb", name="spb")
                    for i in range(2):
                        isl = slice(i * QC, (i + 1) * QC)
                        nc.vector.tensor_tensor(
                            spb[:, isl], sp[:, isl], bt, op=mybir.AluOpType.mult
                        )
                    spb_hold[j] = spb
                # deferred work, emitted AFTER this kt's critical DVE mults,
                # at most ONE DVE op per kt (DVE slack is ~0.7us per kt)
                if norm_st is not None:
                    if kt == 3:
                        norm_recip(norm_st)
                        norm_bcast_dma(norm_st)
                    elif 8 <= kt <= 11:
                        norm_aT2(norm_st, kt - 8)
                    elif 12 <= kt <= 15:
                        oproj_tile(4 * (qc - 1) + (kt - 12))
                        if kt == 15:
                            norm_st = None
            for j in range(2):  # PV for kt = NKT-1
                sprev = spb_hold[j]
                for i in range(2):
                    nc.tensor.matmul(
                        pu[2 * j + i],
                        vaug[:, 2 * j + i, NKT - 1, :],
                        sprev[:, i * QC : (i + 1) * QC],
                        start=False,
                        stop=True,
                    )
            norm_st = norm_part1(qc, pu, last=(qc == NQC - 1))
        # tail: PE-broadcast normalization for the last chunk, then its O-proj
        norm_recip(norm_st)
        norm_bcast_pe(norm_st)
        for h in range(HG):
            norm_aT2(norm_st, h)
        for t in range(12, 16):
            oproj_tile(t, evict_on_act=(t % 2 == 1), dma_on_scalar=(t % 2 == 1))

    nc.compile()
    return nc


_NC = None


def _get_nc():
    global _NC
    if _NC is None:
        _NC = build_program()
    return _NC


def make_in_maps(x, attn_bias, wq, bq, wk, bk, wv, bv, wo, bo):
    x = np.asarray(x, np.float32)
    attn_bias = np.asarray(attn_bias, np.float32)
    # exp(bias)^T per batch, bf16 (shared by the 2 cores of each batch)
    ebTs = [
        np.exp(attn_bias[b, 0].T).astype(BF16NP) for b in range(B)
    ]
    xTs = [np.ascontiguousarray(x[b].T.astype(BF16NP)) for b in range(B)]
    in_maps = []
    for c in range(8):
        b, g = c // 2, c % 2
        sl = slice(g * GD, (g + 1) * GD)
        in_maps.append(
            {
                "xT": xTs[b],
                "ebT": ebTs[b],
                "wq": np.ascontiguousarray(np.asarray(wq, np.float32)[:, sl].astype(BF16NP)),
                "wk": np.ascontiguousarray(np.asarray(wk, np.float32)[:, sl].astype(BF16NP)),
                "wv": np.ascontiguousarray(np.asarray(wv, np.float32)[:, sl].astype(BF16NP)),
                "wo": np.ascontiguousarray(np.asarray(wo, np.float32)[sl, :]),
                "bq": np.ascontiguousarray(np.asarray(bq, np.float32)[sl]),
            }
        )
    return in_maps


def gather_output(results, bo, bv, wo):
    bo = np.asarray(bo, np.float32)
    row = bo + np.asarray(bv, np.float32) @ np.asarray(wo, np.float32)
    out = np.empty((B, N, D), np.float32)
    for b in range(B):
        out[b] = results[2 * b]["out"] + results[2 * b + 1]["out"] + row[None, :]
    return out


def kernel(x, attn_bias, wq, bq, wk, bk, wv, bv, wo, bo, _trace=False):
    nc = _get_nc()
    in_maps = make_in_maps(x, attn_bias, wq, bq, wk, bk, wv, bv, wo, bo)
    res = run_bass_kernel_spmd(nc, in_maps, core_ids=list(range(8)), trace=_trace)
    out = gather_output(res.results, bo, bv, wo)
    if _trace:
        kernel.last_results = res
    return out
